# revision 5
# baseline (speedup 1.0000x reference)
"""Cross-cryptocurrency attention kernel for 8 Trainium2 NeuronCores.

Sharding: 16 (batch, seq-quarter) shards -> core c handles b = c//4,
query rows s in [512*(c%4), 512*(c%4+1)).  Each core computes all 8 heads
and all 9 (query-asset, key-asset) pairs for its query slice, with full
keys/values (S=2048) for its batch, so the output projection is local and
no collectives are needed.

v3 design.  ACT exp (75.5M exps/core -> ~572us incl per-inst overhead) is
the hard floor; everything else is organised to hide under it:
  - host folds biases (bk drops out of softmax exactly; bv/bo fold into
    bo2 = bo + (sum_j bv_j) @ Wo) and pre-packs x/weights in bf16 in the
    exact SBUF layouts, so startup DMAs are few and contiguous.
  - x is transposed by the DMA xbar (dma_start_transpose), zero PE cost.
  - all projections run as [128,256]-output chunk units through a
    dedicated psum bank (psD halves, DVE-memset + start=False chains),
    fully decoupled from the score-psum ring so background work never
    stalls the PE->ACT score pipeline.
  - scores^T[t,s] on PE (lhsT=k^T bf16) in groups (2,3,3,3,3,2); exp on
    ACT PSUM->SBUF bf16.  2-tile groups at the combo boundaries cover the
    next combo's catch-up through the 2-deep score ring.
  - AV with the E tile stationary: out[s,33] accumulates over 16 t-tiles
    at 33 rows each (4x less PE than v-stationary); the ones column of
    [v|1] yields row-sums Z in col 32.  Both combo accumulators pack into
    one psum bank (halves, DVE-memset + start=False).
  - normalize straight off the AV psum: DVE reciprocal + STT mul-add.
  - software pipeline: per combo, the last two AV batches + normalize
    defer into the next combo right after its first score group, so ACT
    never waits at combo boundaries; projection/phase-3 units drip at
    four points per combo with deadlines.
"""

import math
import numpy as np

B = 2
S = 2048
D = 256
H = 8
HD = 32
SQ = 512  # query rows per core
N_CORES = 8
SCALE = 1.0 / math.sqrt(HD)
# 2-tile groups at both ends: boundary exps are 1024 elems, long enough to
# cover the next combo's first score group catching up through the ring.
GROUPS = [(0, 2), (2, 3), (5, 3), (8, 3), (11, 3), (14, 2)]
# exp engine split: ACT handles groups 1-3 natively; DVE handles groups 0,
# 4, 5 via a one-instruction Schraudolph (int16 bits of the bf16 exp,
# written through a bitcast view).  ACT additionally takes the leading
# ACT_COLS[g] columns of a DVE group to fine-balance the engines.
DVE_GROUPS = (0, 4, 5)
ACT_COLS = {4: 512}
# Schraudolph constants: bits = rint(score * SA + SB) as int16 == bf16 bits
# of ~exp(score*SCALE).  SB tuned so the mean relative error is ~0.
SA = 128.0 * math.log2(math.e) * SCALE
SB = 16248.6

_CACHE = {}


def _build():
    from contextlib import ExitStack

    import concourse.bass as bass
    import concourse.mybir as mybir
    import concourse.tile as tile
    from concourse import bacc
    from concourse.masks import make_identity

    f32 = mybir.dt.float32
    bf16 = mybir.dt.bfloat16
    i16 = mybir.dt.int16
    AF = mybir.ActivationFunctionType
    ALU = mybir.AluOpType

    nc = bacc.Bacc("TRN2", target_bir_lowering=False, debug=False)

    x_d = nc.dram_tensor("x", [3, S, D], bf16, kind="ExternalInput").ap()
    # host-packed weights: [p, (a, ty q/k/v, kt, f)] bf16 and [p, (a,dt)] f32
    wpack_d = nc.dram_tensor("wpack", [128, 3 * 3 * 2 * D], bf16, kind="ExternalInput").ap()
    wo_d = nc.dram_tensor("wo", [128, 2 * D], bf16, kind="ExternalInput").ap()
    bq_d = nc.dram_tensor("bqp", [128, 6], f32, kind="ExternalInput").ap()
    bo2_d = nc.dram_tensor("bo2", [D], bf16, kind="ExternalInput").ap()
    out_d = nc.dram_tensor("out", [3, SQ, D], f32, kind="ExternalOutput").ap()

    with tile.TileContext(nc) as tc, ExitStack() as ctx:
        const_p = ctx.enter_context(tc.tile_pool(name="const", bufs=1))
        qkv_p = ctx.enter_context(tc.tile_pool(name="qkv", bufs=1))
        acc_p = ctx.enter_context(tc.tile_pool(name="acc", bufs=1))
        e_p = ctx.enter_context(tc.tile_pool(name="epool", bufs=4))
        sm_p = ctx.enter_context(tc.tile_pool(name="small", bufs=2))
        # PSUM: 3+3 score ring + 1 packed AV accumulators + 1 drip bank
        ps_S = ctx.enter_context(tc.tile_pool(name="psS", bufs=2, space="PSUM"))
        psAB_p = ctx.enter_context(tc.tile_pool(name="psAB", bufs=1, space="PSUM"))
        psD_p = ctx.enter_context(tc.tile_pool(name="psD", bufs=1, space="PSUM"))
        psAB = psAB_p.tile([128, 512], f32, name="psAB")
        psD = psD_p.tile([128, 512], f32, name="psD")

        xT = [qkv_p.tile([128, 2 * S], bf16, tag=f"xT{_}", name=f"xT{_}") for _ in range(3)]
        kT = [qkv_p.tile([128, 2 * S], bf16, tag=f"kT{_}", name=f"kT{_}") for _ in range(3)]
        qT = [qkv_p.tile([128, 2 * SQ], bf16, tag=f"qT{_}", name=f"qT{_}") for _ in range(3)]
        v1 = [qkv_p.tile([128, 16 * (H * 33)], bf16, tag=f"v1_{_}", name=f"v1_{_}") for _ in range(3)]
        out_acc = [acc_p.tile([128, 4 * D], f32, tag=f"oacc{_}", name=f"oacc{_}") for _ in range(3)]

        def dmaT_unit(a, c):
            def run():
                nc.sync.dma_start_transpose(
                    xT[a].rearrange("p (dt s) -> p dt s", dt=2)[:, :, c * 512 : (c + 1) * 512],
                    x_d[a][c * 512 : (c + 1) * 512, :],
                )
            return run

        # ---- startup DMAs ordered for minimum time-to-first-score-group:
        # SP queue carries x chunk 0 + asset-0 weights; the idle ACT queue
        # dispatches the small/late constants in parallel ----
        dmaT_unit(0, 0)()
        wsb = const_p.tile([128, 3 * 3 * 2 * D], bf16)
        nc.gpsimd.dma_start(wsb[:, 0 : 3 * 2 * D], wpack_d[:, 0 : 3 * 2 * D])
        bq_sb = const_p.tile([128, 6], f32)
        nc.gpsimd.dma_start(bq_sb[:], bq_d)
        for c in range(1, 4):
            dmaT_unit(0, c)()
        nc.sync.dma_start(wsb[:, 3 * 2 * D :], wpack_d[:, 3 * 2 * D :])
        wo_sb = const_p.tile([128, 2 * D], bf16)
        nc.sync.dma_start(wo_sb[:], wo_d)
        bo2_row = const_p.tile([1, D], bf16)
        nc.sync.dma_start(bo2_row[:], bo2_d[None, :])
        ident = const_p.tile([128, 128], f32)
        make_identity(nc, ident[:])
        onesb = const_p.tile([1, 128], bf16)
        nc.gpsimd.memset(onesb[:], 1.0)

        # ---- drip bank: [128,256] halves.  Chains open with start=True:
        # the bank-wide zero region is applied lazily (zero-on-next-matmul-
        # touch), so the other half's finished result stays readable for its
        # pending DVE copy; chains themselves are sequential in PE order. ----
        dctr = [0]

        def dhalf():
            hh = dctr[0] & 1
            dctr[0] += 1
            return psD[:, hh * 256 : (hh + 1) * 256]

        WT_Q, WT_K, WT_V = 0, 1, 2

        def wcol(a, ty, kt, off, width):
            base = a * (3 * 2 * D) + ty * (2 * D) + kt * D + off
            return wsb[:, base : base + width]

        def k_unit(a, dt, n):  # kT[a] cols [dt*S + 256n, +256)
            def run():
                reg = dhalf()
                for kt in range(2):
                    nc.tensor.matmul(
                        reg,
                        wcol(a, WT_K, kt, dt * 128, 128),
                        xT[a][:, kt * S + n * 256 : kt * S + (n + 1) * 256],
                        start=(kt == 0), stop=(kt == 1), skip_group_check=True,
                    )
                nc.vector.tensor_copy(
                    kT[a][:, dt * S + n * 256 : dt * S + (n + 1) * 256], reg
                )
            return run

        def q_unit(a, dt, n):  # qT[a] cols [dt*SQ + 256n, +256)
            def run():
                reg = dhalf()
                for kt in range(2):
                    nc.tensor.matmul(
                        reg,
                        wcol(a, WT_Q, kt, dt * 128, 128),
                        xT[a][:, kt * S + n * 256 : kt * S + (n + 1) * 256],
                        start=(kt == 0), stop=(kt == 1), skip_group_check=True,
                    )
                nc.vector.tensor_scalar_add(
                    qT[a][:, dt * SQ + n * 256 : dt * SQ + (n + 1) * 256],
                    reg,
                    bq_sb[:, a * 2 + dt : a * 2 + dt + 1],
                )
            return run

        def v_unit(a, st):
            def run():
                reg = dhalf()
                for kt in range(2):
                    nc.tensor.matmul(
                        reg,
                        xT[a][:, kt * S + st * 128 : kt * S + (st + 1) * 128],
                        wcol(a, WT_V, kt, 0, D),
                        start=(kt == 0), stop=(kt == 1), skip_group_check=True,
                    )
                dst = v1[a][
                    :, st * (H * 33) : (st + 1) * (H * 33)
                ].rearrange("p (h x) -> p h x", x=33)[:, :, 0:32]
                nc.vector.tensor_copy(dst, reg.rearrange("p (h x) -> p h x", x=32))
            return run

        def ones_unit(a):
            def run():
                nc.gpsimd.memset(
                    v1[a].rearrange("p (t h x) -> p (t h) x", h=H, x=33)[:, :, 32:33],
                    1.0,
                )
            return run

        # ======== Phase 2: one (i, j, h) combo ========
        def emit_av(eg, t0, glen, j, h, reg):
            for u in range(glen):
                tt = t0 + u
                for k in range(4):
                    nc.tensor.matmul(
                        reg[:, k * 33 : (k + 1) * 33],
                        eg[:, u * 512 + k * 128 : u * 512 + (k + 1) * 128],
                        v1[j][:, tt * (H * 33) + h * 33 : tt * (H * 33) + (h + 1) * 33],
                        start=(tt == 0 and k == 0),
                        stop=(tt == 15 and k == 3),
                        skip_group_check=True,
                    )

        def norm_unit(i, j, h, reg):
            # DVE: drain the AV psum half to SBUF + reciprocal of the Z
            # column; Pool: the 4 normalize/accumulate passes (SBUF-only).
            def run():
                avs = sm_p.tile([128, 132], f32, tag="avs", name="avs")
                nc.vector.tensor_copy(avs[:], reg)
                rr4 = sm_p.tile([128, 4], f32, tag="rr", name="rr")
                nc.vector.reciprocal_approx_fast(
                    rr4[:],
                    avs[:].rearrange("p (k x) -> p k x", x=33)[:, :, 32],
                )
                if j == 0:
                    for k in range(4):
                        nc.gpsimd.tensor_scalar_mul(
                            out_acc[i][:, k * D + h * 32 : k * D + (h + 1) * 32],
                            avs[:, k * 33 : k * 33 + 32],
                            rr4[:, k : k + 1],
                        )
                else:
                    # Pool has no scalar_tensor_tensor: stage the normalized
                    # chunk in tmp, then one strided add into out_acc.
                    tmp = sm_p.tile([128, 128], f32, tag="ntmp", name="ntmp")
                    for k in range(4):
                        nc.gpsimd.tensor_scalar_mul(
                            tmp[:, k * 32 : (k + 1) * 32],
                            avs[:, k * 33 : k * 33 + 32],
                            rr4[:, k : k + 1],
                        )
                    oav = out_acc[i].rearrange("p (k d) -> p k d", d=D)[
                        :, :, h * 32 : (h + 1) * 32
                    ]
                    nc.gpsimd.tensor_tensor(
                        oav, oav, tmp[:].rearrange("p (k r) -> p k r", r=32), op=ALU.add
                    )
            return run

        def combo(ci, i, j, h, tail, drip, pre_sc=None, pre_av=None, pre_tail=None):
            hp = 32 * (h % 4)
            hc = h // 4
            reg = psAB[:, (ci % 2) * 256 : (ci % 2) * 256 + 132]
            egs = []

            def sc(gi):
                t0, glen = GROUPS[gi]
                psS = ps_S.tile([128, glen * 512], f32, tag="psS", name="ps2")
                for u in range(glen):
                    tt = t0 + u
                    nc.tensor.matmul(
                        psS[:, u * 512 : (u + 1) * 512],
                        kT[j][hp : hp + 32, hc * S + tt * 128 : hc * S + (tt + 1) * 128],
                        qT[i][hp : hp + 32, hc * SQ : (hc + 1) * SQ],
                        start=True,
                        stop=True,
                        tile_position=(hp, 0),
                    )
                eg = e_p.tile([128, 3 * 512], bf16, tag="eg", name="eg")
                n = glen * 512
                if gi in DVE_GROUPS:
                    ac = ACT_COLS.get(gi, 0)
                    if ac:
                        nc.scalar.activation(
                            eg[:, 0:ac], psS[:, 0:ac], AF.Exp, scale=SCALE
                        )
                    nc.vector.tensor_scalar(
                        eg[:, ac:n].bitcast(i16),
                        psS[:, ac:n],
                        SA,
                        SB,
                        op0=ALU.mult,
                        op1=ALU.add,
                    )
                else:
                    nc.scalar.activation(eg[:, 0:n], psS[:], AF.Exp, scale=SCALE)
                egs.append((eg, t0, glen))

            def hook(d, gi):
                if d and gi in d:
                    for u in d[gi]:
                        u()

            hook(pre_sc, 0)
            sc(0)
            if pre_tail:
                for u in pre_tail:
                    u()
            for t in tail:
                t()
            hook(pre_sc, 1)
            drip(1)
            sc(1)
            hook(pre_av, 0)
            drip(1)
            emit_av(*egs[0], j, h, reg)
            hook(pre_sc, 2)
            drip(1)
            sc(2)
            hook(pre_av, 1)
            drip(1)
            emit_av(*egs[1], j, h, reg)
            hook(pre_sc, 3)
            drip(1)
            sc(3)
            hook(pre_av, 2)
            drip(1)
            emit_av(*egs[2], j, h, reg)
            hook(pre_sc, 4)
            drip(1)
            sc(4)
            hook(pre_av, 3)
            drip(1)
            emit_av(*egs[3], j, h, reg)
            hook(pre_sc, 5)
            drip(1)
            sc(5)
            return [
                lambda: emit_av(*egs[4], j, h, reg),
                lambda: emit_av(*egs[5], j, h, reg),
                norm_unit(i, j, h, reg),
            ]

        # ======== Phase 3: output projection for one asset, as units ======
        aT = [acc_p.tile([128, 2 * SQ], bf16, tag=f"aT{_}", name=f"aT{_}") for _ in range(3)]

        def t_unit(i, dt, half):  # transpose out_acc block -> aT bf16
            def run():
                reg = dhalf()
                for k in range(2):
                    st = 2 * half + k
                    nc.tensor.matmul(
                        reg[:, k * 128 : (k + 1) * 128],
                        out_acc[i][:, st * D + dt * 128 : st * D + dt * 128 + 128],
                        ident[:],
                        is_transpose=True,
                        start=(k == 0), stop=True, skip_group_check=True,
                    )
                nc.vector.tensor_copy(
                    aT[i][:, dt * SQ + half * 256 : dt * SQ + (half + 1) * 256], reg
                )
            return run

        def p_unit(i, st):
            def run():
                reg = dhalf()
                for dt in range(2):
                    nc.tensor.matmul(
                        reg,
                        aT[i][:, dt * SQ + st * 128 : dt * SQ + (st + 1) * 128],
                        wo_sb[:, dt * D : (dt + 1) * D],
                        start=(dt == 0), stop=False, skip_group_check=True,
                    )
                nc.tensor.matmul(
                    reg,
                    onesb[0:1, 0:128],
                    bo2_row[0:1, :],
                    start=False, stop=True, skip_group_check=True,
                )
                ot = sm_p.tile([128, D], f32, tag="ot", name="ot")
                nc.vector.tensor_copy(ot[:], reg)
                nc.sync.dma_start(
                    out_d[i].rearrange("(st p) d -> st p d", p=128)[st], ot[:]
                )
            return run

        def ph3_units(i):
            return [t_unit(i, dt, half) for dt in range(2) for half in range(2)] + [
                p_unit(i, st) for st in range(4)
            ]

        # ======== Emission schedule ========
        # startup prefix: just enough for combo (0,0,0) group 0
        q_unit(0, 0, 0)()
        q_unit(0, 0, 1)()
        k_unit(0, 0, 0)()
        ones_unit(0)()

        # combo-0/1 custom placement: asset-0 dt0 k-units and v-units land
        # exactly where the score groups / AV batches need them
        c0_pre_sc = {
            1: [k_unit(0, 0, 1), k_unit(0, 0, 2)],
            2: [k_unit(0, 0, 3)],
            3: [k_unit(0, 0, 4), k_unit(0, 0, 5)],
            4: [k_unit(0, 0, 6)],
            5: [k_unit(0, 0, 7)],
        }
        c0_pre_av = {
            0: [v_unit(0, 0), v_unit(0, 1)],
            1: [v_unit(0, 2), v_unit(0, 3), v_unit(0, 4)],
            2: [v_unit(0, 5), v_unit(0, 6), v_unit(0, 7)],
            3: [v_unit(0, 8), v_unit(0, 9), v_unit(0, 10)],
        }
        c1_pre_tail = [v_unit(0, st) for st in range(11, 16)]

        # deadline-tagged drip queue
        dripq = []
        for dt in range(2):
            for n in range(2):
                dripq.append((q_unit(0, 1, n), 0))
        for n in range(8):
            dripq.append((k_unit(0, 1, n), 0))
        for a in (1, 2):
            for c in range(4):
                dripq.append((dmaT_unit(a, c), 0))
            dripq.append((ones_unit(a), 0))
            for dt in range(2):
                for n in range(2):
                    dripq.append((q_unit(a, dt, n), 0))
            for dt in range(2):
                for n in range(8):
                    dripq.append((k_unit(a, dt, n), 0))
            for st in range(16):
                dripq.append((v_unit(a, st), 0))
        for u in ph3_units(0):
            dripq.append((u, 24))
        for u in ph3_units(1):
            dripq.append((u, 48))

        ci_box = [0]

        def drip(budget):
            while dripq and budget > 0 and dripq[0][1] <= ci_box[0]:
                dripq.pop(0)[0]()
                budget -= 1

        tail = []
        ci = 0
        for i in range(3):
            for j in range(3):
                for h in range(H):
                    ci_box[0] = ci
                    combo_kw = {}
                    if ci == 0:
                        combo_kw = dict(pre_sc=c0_pre_sc, pre_av=c0_pre_av)
                    elif ci == 1:
                        combo_kw = dict(pre_tail=c1_pre_tail)
                    tail = combo(ci, i, j, h, tail, drip, **combo_kw)
                    ci += 1
        for t in tail:
            t()
        while dripq:
            dripq.pop(0)[0]()
        for u in ph3_units(2):
            u()
    nc.compile()
    return nc


def kernel(x_btc, x_eth, x_sol, Wq, bq, Wk, bk, Wv, bv, Wo, bo):
    import ml_dtypes
    from concourse.bass_utils import run_bass_kernel_spmd

    if "nc" not in _CACHE:
        _CACHE["nc"] = _build()
    nc = _CACHE["nc"]

    bff = ml_dtypes.bfloat16
    xs = [np.asarray(t, dtype=np.float32) for t in (x_btc, x_eth, x_sol)]
    # fold v-bias and o-bias: out = attn @ Wo + (sum_j bv_j) @ Wo + bo
    bo2 = (np.asarray(bo, np.float64)
           + np.asarray(bv, np.float64).sum(0) @ np.asarray(Wo, np.float64))
    # weight pack [p, (a, ty, kt, f)]: wpack[p, a,ty,kt,f] = W_ty[a, kt*128+p, f]
    wqkv = np.stack([np.asarray(W, np.float32) for W in (Wq, Wk, Wv)], axis=1)
    wpack = np.ascontiguousarray(
        wqkv.reshape(3, 3, 2, 128, D).transpose(3, 0, 1, 2, 4).reshape(128, 3 * 3 * 2 * D)
    ).astype(bff)
    wo_p = np.ascontiguousarray(
        np.asarray(Wo, np.float32).reshape(2, 128, D).transpose(1, 0, 2).reshape(128, 2 * D)
    ).astype(bff)
    bq_p = np.ascontiguousarray(
        np.asarray(bq, np.float32).reshape(3, 2, 128).transpose(2, 0, 1).reshape(128, 6)
    )
    common = {
        "wpack": wpack,
        "wo": wo_p,
        "bqp": bq_p,
        "bo2": bo2.astype(np.float32).astype(bff),
    }
    in_maps = []
    for c in range(N_CORES):
        b, sq = c // 4, c % 4
        # Roll the sequence so this core's query quarter sits at rows [0:512)
        # (the kernel always projects q from rows 0:512).  k/v see the rolled
        # full sequence, which is fine: softmax+sum over the key axis is
        # permutation-invariant.
        xq = np.stack(
            [np.roll(xs[i][b], -sq * SQ, axis=0) for i in range(3)]
        ).astype(bff)
        in_maps.append({"x": np.ascontiguousarray(xq), **common})
    import os
    res = run_bass_kernel_spmd(
        nc, in_maps, core_ids=list(range(N_CORES)),
        trace=bool(os.environ.get("BASS_TRACE")),
    )
    _CACHE["last_res"] = res

    outs = [np.empty((B, S, D), np.float32) for _ in range(3)]
    for c in range(N_CORES):
        b, sq = c // 4, c % 4
        o = res.results[c]["out"]
        for i in range(3):
            outs[i][b, sq * SQ : (sq + 1) * SQ] = o[i]
    return tuple(outs)


if __name__ == "__main__":
    import reference

    inp = reference.setup_inputs()
    inp = {k: np.asarray(v) for k, v in inp.items()}
    got = kernel(**inp)
    exp = reference.reference(**inp)
    for i in range(3):
        g, e = np.asarray(got[i]), np.asarray(exp[i])
        err = np.abs(g - e).max() / np.abs(e).max()
        print(f"out[{i}] rel err {err:.3e}")



# revision 56
# speedup vs baseline: 1.5571x; 1.5571x over previous
"""Cross-cryptocurrency attention kernel for 8 Trainium2 NeuronCores.

Sharding: 16 (batch, seq-quarter) shards -> core c handles b = c//4,
query rows s in [512*(c%4), 512*(c%4+1)).  Each core computes all 8 heads
and all 9 (query-asset, key-asset) pairs for its query slice, with full
keys/values (S=2048) for its batch, so the output projection is local and
no collectives are needed.

v6 design.  The PSUM->SBUF drain (75.5M exps + projection copies per
core) is the floor; only ACT and DVE can read PSUM, so the exp work is
split across BOTH engines and everything else is organised to keep the
three-engine pipeline latency-tolerant:
  - host folds biases (bk drops out of softmax exactly; bv/bo fold into
    bo2 = bo + (sum_j bv_j) @ Wo) and pre-packs x/weights in bf16.
  - x is transposed by the DMA xbar (dma_start_transpose), zero PE cost.
  - scores^T[t,s] on PE in 8 groups of 2 key-tiles through a 3-deep psum
    ring (3x1024 f32 + AV bank + drip bank = the full 16KB/partition).
    Ring depth 3 hides the PE->exp->PE hop latency (access ack + sem
    propagation) that serialized a 2-deep ring.
  - exp split ~56/44 between ACT (native Exp, psum->sbuf bf16) and DVE
    (one-instruction Schraudolph: bits = rint(score*SA + SB) as int16
    written through a bitcast view of the bf16 eg tile; ~1.8% rms weight
    noise on the DVE share, rel err ~7e-3 end to end).  Engines
    interleave around the ring so neither forms a long serial chain.
  - AV with the E tile stationary accumulates into a packed psum bank
    (2 combo halves, start=True lazy bank zero); the ones column of
    [v|1] yields row-sums Z in col 32.  AV emission lags scores by two
    groups; the last two AV batches + normalize defer into the next
    combo.
  - normalize: DVE drains the AV psum half + reciprocal; the 4 per-chunk
    scale/accumulate passes run on the otherwise-idle Pool engine
    (SBUF-only, Pool cannot read PSUM).
  - background projections run as merged double-width units (two psD
    half-chains + one 512-col copy) dripped between score groups with
    deadlines, halving DVE copy instruction count.
  - warmup: asset-0 k/v projections pipeline through the 3-deep score
    ring before combo 0 (the serial psD bank costs ~1.4us per unit in
    chain->sem->copy->sem latency), cutting combo 0 from ~36us to ~17us.
"""

import math
import numpy as np

B = 2
S = 2048
D = 256
H = 8
HD = 32
SQ = 512  # query rows per core
N_CORES = 8
SCALE = 1.0 / math.sqrt(HD)
# 8 groups of 2 key-tiles through a 3-deep psum ring (3x1024 f32 = 12KB +
# psAB 2KB + psD 2KB = the full 16KB/partition).  Ring depth 3 gives each
# group's exp ~2 group-periods of slack, hiding the PE->exp->PE hop
# latency (psum/sbuf access ack + semaphore propagation) that made a
# 2-deep ring serialize.
GROUPS = [(2 * g, 2) for g in range(8)]
# exp engine split: ACT runs native exp on groups with ACT_COLS=full;
# DVE runs a one-instruction Schraudolph (int16 bits of the bf16 exp
# through a bitcast view) on the rest.  Engines are interleaved around
# the ring so neither forms a long serial chain.
ACT_COLS = [1024, 0, 928, 1024, 0, 1024, 0, 960]
# Schraudolph constants: bits = rint(score * SA + SB) as int16 == bf16 bits
# of ~exp(score*SCALE).  SB tuned so the mean relative error is ~0.
SA = 128.0 * math.log2(math.e) * SCALE
SB = 16248.6

_CACHE = {}


def _build():
    from contextlib import ExitStack

    import concourse.bass as bass
    import concourse.mybir as mybir
    import concourse.tile as tile
    from concourse import bacc
    from concourse.masks import make_identity

    f32 = mybir.dt.float32
    bf16 = mybir.dt.bfloat16
    i16 = mybir.dt.int16
    AF = mybir.ActivationFunctionType
    ALU = mybir.AluOpType

    nc = bacc.Bacc("TRN2", target_bir_lowering=False, debug=False)

    x_d = nc.dram_tensor("x", [3, S, D], bf16, kind="ExternalInput").ap()
    # host-packed weights: [p, (a, ty q/k/v, kt, f)] bf16 and [p, (a,dt)] f32
    wpack_d = nc.dram_tensor("wpack", [128, 3 * 3 * 2 * D], bf16, kind="ExternalInput").ap()
    wo_d = nc.dram_tensor("wo", [128, 2 * D], bf16, kind="ExternalInput").ap()
    bq_d = nc.dram_tensor("bqp", [128, 6], f32, kind="ExternalInput").ap()
    bo2_d = nc.dram_tensor("bo2", [D], bf16, kind="ExternalInput").ap()
    out_d = nc.dram_tensor("out", [3, SQ, D], f32, kind="ExternalOutput").ap()

    with tile.TileContext(nc) as tc, ExitStack() as ctx:
        const_p = ctx.enter_context(tc.tile_pool(name="const", bufs=1))
        qkv_p = ctx.enter_context(tc.tile_pool(name="qkv", bufs=1))
        acc_p = ctx.enter_context(tc.tile_pool(name="acc", bufs=1))
        e_p = ctx.enter_context(tc.tile_pool(name="epool", bufs=7))
        sm_p = ctx.enter_context(tc.tile_pool(name="small", bufs=3))
        # PSUM: 3-deep score ring + 1 packed AV accumulators + 1 drip bank
        ps_S = ctx.enter_context(tc.tile_pool(name="psS", bufs=3, space="PSUM"))
        psAB_p = ctx.enter_context(tc.tile_pool(name="psAB", bufs=1, space="PSUM"))
        psD_p = ctx.enter_context(tc.tile_pool(name="psD", bufs=1, space="PSUM"))
        psAB = psAB_p.tile([128, 512], f32, name="psAB")
        psD = psD_p.tile([128, 512], f32, name="psD")

        xT = [qkv_p.tile([128, 2 * S], bf16, tag=f"xT{_}", name=f"xT{_}") for _ in range(3)]
        kT = [qkv_p.tile([128, 2 * S], bf16, tag=f"kT{_}", name=f"kT{_}") for _ in range(3)]
        qT = [qkv_p.tile([128, 2 * SQ], bf16, tag=f"qT{_}", name=f"qT{_}") for _ in range(3)]
        v1 = [qkv_p.tile([128, 16 * (H * 33)], bf16, tag=f"v1_{_}", name=f"v1_{_}") for _ in range(3)]
        out_acc = [acc_p.tile([128, 4 * D], f32, tag=f"oacc{_}", name=f"oacc{_}") for _ in range(3)]

        def dmaT_unit(a, c, eng=None):
            def run():
                (eng or nc.sync).dma_start_transpose(
                    xT[a].rearrange("p (dt s) -> p dt s", dt=2)[:, :, c * 512 : (c + 1) * 512],
                    x_d[a][c * 512 : (c + 1) * 512, :],
                )
            return run

        # ---- startup DMAs ordered for minimum time-to-first-score-group:
        # SP queue carries x chunk 0 + asset-0 weights; the idle ACT queue
        # dispatches the small/late constants in parallel ----
        # asset-0 x transposes fan out over the three HWDGE queues (SP,
        # DVE, ACT) so combo 0 is not gated on one serialized queue; the
        # compute engines are idle this early, so the dispatch cost is free.
        dmaT_unit(0, 0)()
        wsb = const_p.tile([128, 3 * 3 * 2 * D], bf16)
        nc.gpsimd.dma_start(wsb[:, 0 : 3 * 2 * D], wpack_d[:, 0 : 3 * 2 * D])
        bq_sb = const_p.tile([128, 6], f32)
        nc.gpsimd.dma_start(bq_sb[:], bq_d)
        dmaT_unit(0, 1)()
        dmaT_unit(0, 2)()
        dmaT_unit(0, 3)()
        nc.sync.dma_start(wsb[:, 3 * 2 * D :], wpack_d[:, 3 * 2 * D :])
        wo_sb = const_p.tile([128, 2 * D], bf16)
        nc.sync.dma_start(wo_sb[:], wo_d)
        bo2_row = const_p.tile([1, D], bf16)
        nc.sync.dma_start(bo2_row[:], bo2_d[None, :])
        ident = const_p.tile([128, 128], f32)
        make_identity(nc, ident[:])
        onesb = const_p.tile([1, 128], bf16)
        nc.gpsimd.memset(onesb[:], 1.0)

        # ---- drip bank: [128,256] halves.  Chains open with start=True:
        # the bank-wide zero region is applied lazily (zero-on-next-matmul-
        # touch), so the other half's finished result stays readable for its
        # pending DVE copy; chains themselves are sequential in PE order. ----
        dctr = [0]

        def dhalf():
            hh = dctr[0] & 1
            dctr[0] += 1
            return psD[:, hh * 256 : (hh + 1) * 256]

        WT_Q, WT_K, WT_V = 0, 1, 2

        def wcol(a, ty, kt, off, width):
            base = a * (3 * 2 * D) + ty * (2 * D) + kt * D + off
            return wsb[:, base : base + width]

        # merged drip units: two psD half-chains back to back, then one
        # double-width copy.  Every unit consumes exactly two halves, so
        # dhalf() parity stays aligned and the pair is always contiguous.
        def ring_halves():
            tile_ = ps_S.tile([128, 2 * 512], f32, tag="psS", name="ps2")
            return [tile_[:, 0:256], tile_[:, 256:512]], tile_[:, 0:512]

        def k_unit2(a, dt, np_, ring=False, act_copy=False):
            # kT[a] cols [dt*S + 512*np_, +512)
            def run():
                if ring:
                    regs, src = ring_halves()
                else:
                    regs, src = [dhalf(), dhalf()], psD[:, 0:512]
                for w in range(2):
                    n = 2 * np_ + w
                    for kt in range(2):
                        nc.tensor.matmul(
                            regs[w],
                            wcol(a, WT_K, kt, dt * 128, 128),
                            xT[a][:, kt * S + n * 256 : kt * S + (n + 1) * 256],
                            start=(kt == 0), stop=(kt == 1), skip_group_check=True,
                        )
                dstk = kT[a][:, dt * S + np_ * 512 : dt * S + (np_ + 1) * 512]
                if act_copy:
                    nc.scalar.activation(dstk, src, AF.Copy, scale=1.0)
                else:
                    nc.vector.tensor_copy(dstk, src)
            return run

        def q_unit2(a, dt):  # qT[a] cols [dt*SQ, +512)
            def run():
                regs = [dhalf(), dhalf()]
                for w in range(2):
                    for kt in range(2):
                        nc.tensor.matmul(
                            regs[w],
                            wcol(a, WT_Q, kt, dt * 128, 128),
                            xT[a][:, kt * S + w * 256 : kt * S + (w + 1) * 256],
                            start=(kt == 0), stop=(kt == 1), skip_group_check=True,
                        )
                nc.vector.tensor_scalar_add(
                    qT[a][:, dt * SQ : dt * SQ + 512],
                    psD[:, 0:512],
                    bq_sb[:, a * 2 + dt : a * 2 + dt + 1],
                )
            return run

        def v_unit2(a, stp, ring=False):  # v1[a] tiles {2stp, 2stp+1}
            def run():
                if ring:
                    regs, src = ring_halves()
                else:
                    regs, src = [dhalf(), dhalf()], psD[:, 0:512]
                for w in range(2):
                    st = 2 * stp + w
                    for kt in range(2):
                        nc.tensor.matmul(
                            regs[w],
                            xT[a][:, kt * S + st * 128 : kt * S + (st + 1) * 128],
                            wcol(a, WT_V, kt, 0, D),
                            start=(kt == 0), stop=(kt == 1), skip_group_check=True,
                        )
                dst = v1[a][
                    :, 2 * stp * (H * 33) : (2 * stp + 2) * (H * 33)
                ].rearrange("p (st h x) -> p (st h) x", x=33, h=H)[:, :, 0:32]
                nc.vector.tensor_copy(
                    dst, src.rearrange("p (g x) -> p g x", x=32)
                )
            return run

        def ones_unit(a):
            def run():
                nc.gpsimd.memset(
                    v1[a].rearrange("p (t h x) -> p (t h) x", h=H, x=33)[:, :, 32:33],
                    1.0,
                )
            return run

        # ======== Phase 2: one (i, j, h) combo ========
        def emit_av(eg, t0, glen, j, h, reg):
            for u in range(glen):
                tt = t0 + u
                for k in range(4):
                    nc.tensor.matmul(
                        reg[:, k * 33 : (k + 1) * 33],
                        eg[:, u * 512 + k * 128 : u * 512 + (k + 1) * 128],
                        v1[j][:, tt * (H * 33) + h * 33 : tt * (H * 33) + (h + 1) * 33],
                        start=(tt == 0 and k == 0),
                        stop=(tt == 15 and k == 3),
                        skip_group_check=True,
                    )

        def norm_unit(i, j, h, reg):
            # DVE: drain the AV psum half to SBUF + reciprocal of the Z
            # column; Pool: the 4 normalize/accumulate passes (SBUF-only).
            def run():
                avs = sm_p.tile([128, 132], f32, tag="avs", name="avs")
                nc.vector.tensor_copy(avs[:], reg)
                rr4 = sm_p.tile([128, 4], f32, tag="rr", name="rr")
                nc.vector.reciprocal_approx_fast(
                    rr4[:],
                    avs[:].rearrange("p (k x) -> p k x", x=33)[:, :, 32],
                )
                if j == 0:
                    for k in range(4):
                        nc.gpsimd.tensor_scalar_mul(
                            out_acc[i][:, k * D + h * 32 : k * D + (h + 1) * 32],
                            avs[:, k * 33 : k * 33 + 32],
                            rr4[:, k : k + 1],
                        )
                else:
                    # Pool has no scalar_tensor_tensor: stage the normalized
                    # chunk in tmp, then one strided add into out_acc.
                    tmp = sm_p.tile([128, 128], f32, tag="ntmp", name="ntmp")
                    for k in range(4):
                        nc.gpsimd.tensor_scalar_mul(
                            tmp[:, k * 32 : (k + 1) * 32],
                            avs[:, k * 33 : k * 33 + 32],
                            rr4[:, k : k + 1],
                        )
                    oav = out_acc[i].rearrange("p (k d) -> p k d", d=D)[
                        :, :, h * 32 : (h + 1) * 32
                    ]
                    nc.gpsimd.tensor_tensor(
                        oav, oav, tmp[:].rearrange("p (k r) -> p k r", r=32), op=ALU.add
                    )
            return run

        def combo(ci, i, j, h, tail, drip, pre_sc=None, pre_av=None, pre_tail=None):
            hp = 32 * (h % 4)
            hc = h // 4
            reg = psAB[:, (ci % 2) * 256 : (ci % 2) * 256 + 132]
            egs = []

            def sc(gi):
                t0, glen = GROUPS[gi]
                psS = ps_S.tile([128, 2 * 512], f32, tag="psS", name="ps2")
                for u in range(glen):
                    tt = t0 + u
                    nc.tensor.matmul(
                        psS[:, u * 512 : (u + 1) * 512],
                        kT[j][hp : hp + 32, hc * S + tt * 128 : hc * S + (tt + 1) * 128],
                        qT[i][hp : hp + 32, hc * SQ : (hc + 1) * SQ],
                        start=True,
                        stop=True,
                        tile_position=(hp, 0),
                    )
                eg = e_p.tile([128, 2 * 512], bf16, tag="eg", name="eg")
                n = glen * 512
                ac = min(ACT_COLS[gi], n)
                if ac:
                    nc.scalar.activation(
                        eg[:, 0:ac], psS[:, 0:ac], AF.Exp, scale=SCALE
                    )
                if ac < n:
                    nc.vector.tensor_scalar(
                        eg[:, ac:n].bitcast(i16),
                        psS[:, ac:n],
                        SA,
                        SB,
                        op0=ALU.mult,
                        op1=ALU.add,
                    )
                egs.append((eg, t0, glen))

            def hook(d, gi):
                if d and gi in d:
                    for u in d[gi]:
                        u()

            NG = len(GROUPS)
            hook(pre_sc, 0)
            sc(0)
            if pre_tail:
                for u in pre_tail:
                    u()
            for t in tail[:2]:
                t()
            for g in range(1, NG):
                hook(pre_sc, g)
                drip(1)
                sc(g)
                if g == 1:
                    # prev combo's normalize: emitted after sc(1) so the
                    # DVE avs-copy/recip queue behind this combo's first
                    # DVE exp slice, not in front of it.
                    for t in tail[2:]:
                        t()
                if g <= NG - 2:
                    hook(pre_av, g - 1)
                if g >= 2:
                    emit_av(*egs[g - 2], j, h, reg)
            return [
                lambda: emit_av(*egs[NG - 2], j, h, reg),
                lambda: emit_av(*egs[NG - 1], j, h, reg),
                norm_unit(i, j, h, reg),
            ]

        # ======== Phase 3: output projection for one asset, as units ======
        aT = [acc_p.tile([128, 2 * SQ], bf16, tag=f"aT{_}", name=f"aT{_}") for _ in range(3)]

        def t_unit2(i, dt):  # transpose out_acc -> aT[i] cols [dt*SQ, +512)
            def run():
                regs = [dhalf(), dhalf()]
                for w in range(2):
                    for k in range(2):
                        st = 2 * w + k
                        nc.tensor.matmul(
                            regs[w][:, k * 128 : (k + 1) * 128],
                            out_acc[i][:, st * D + dt * 128 : st * D + dt * 128 + 128],
                            ident[:],
                            is_transpose=True,
                            start=(k == 0), stop=True, skip_group_check=True,
                        )
                nc.vector.tensor_copy(
                    aT[i][:, dt * SQ : dt * SQ + 512], psD[:, 0:512]
                )
            return run

        def p_unit2(i, stp):  # output rows [256*stp, +256)
            def run():
                regs = [dhalf(), dhalf()]
                for w in range(2):
                    st = 2 * stp + w
                    for dt in range(2):
                        nc.tensor.matmul(
                            regs[w],
                            aT[i][:, dt * SQ + st * 128 : dt * SQ + (st + 1) * 128],
                            wo_sb[:, dt * D : (dt + 1) * D],
                            start=(dt == 0), stop=False, skip_group_check=True,
                        )
                    nc.tensor.matmul(
                        regs[w],
                        onesb[0:1, 0:128],
                        bo2_row[0:1, :],
                        start=False, stop=True, skip_group_check=True,
                    )
                ot = sm_p.tile([128, 2 * D], f32, tag="ot", name="ot")
                nc.vector.tensor_copy(ot[:], psD[:, 0:512])
                nc.sync.dma_start(
                    out_d[i][2 * stp * 128 : (2 * stp + 2) * 128, :].rearrange(
                        "(st p) d -> p st d", p=128
                    ),
                    ot[:].rearrange("p (st d) -> p st d", d=D),
                )
            return run

        def ph3_units(i, dts=(0, 1)):
            return [t_unit2(i, dt) for dt in dts] + [
                p_unit2(i, stp) for stp in range(2)
            ]

        # ======== Emission schedule ========
        # startup prelude: asset-0 dt0 k and the first v tiles go through
        # the 3-deep score ring (3 units in flight) instead of the serial
        # psD bank, so combo 0 starts fully fed.
        q_unit2(0, 0)()
        ones_unit(0)()
        for np_ in range(4):
            k_unit2(0, 0, np_, ring=True)()
        for stp in range(4):
            v_unit2(0, stp, ring=True)()

        # combo-0/1 custom placement: asset-0 dt0 k-units and v-units land
        # exactly where the score groups / AV batches need them.  Score
        # group g consumes kT cols covered by k_unit2(0, 0, g//2); AV
        # batch k consumes v-tiles {2k, 2k+1} = v_unit2(0, k).
        c0_pre_sc = {6: [v_unit2(0, 7)]}
        c0_pre_av = {3: [v_unit2(0, 4)], 4: [v_unit2(0, 5)], 5: [v_unit2(0, 6)]}
        c1_pre_tail = []

        # deadline-tagged drip queue: deadlines track first consumption
        # (asset a's keys/values at ci=8a*... kT dt0 ci=8+24a, dt1 +4,
        # qT at ci=24a) so warmup combos are not overloaded.
        dripq = []
        dripq.append((q_unit2(0, 1), 0))
        for np_ in range(4):
            dripq.append((k_unit2(0, 1, np_), min(np_, 2)))
        for a in (1, 2):
            for c in range(4):
                dripq.append((dmaT_unit(a, c), 0))
            dripq.append((ones_unit(a), 0))
            for dt in range(2):
                dripq.append((q_unit2(a, dt), 0))
            for dt in range(2):
                for np_ in range(4):
                    dripq.append((k_unit2(a, dt, np_), 0))
            for stp in range(8):
                dripq.append((v_unit2(a, stp), 0))
        for u in ph3_units(0):
            dripq.append((u, 24))
        for u in ph3_units(1):
            dripq.append((u, 48))
        dripq.append((t_unit2(2, 0), 69))

        ci_box = [0]

        def drip(budget):
            while dripq and budget > 0 and dripq[0][1] <= ci_box[0]:
                dripq.pop(0)[0]()
                budget -= 1

        tail = []
        ci = 0
        for i in range(3):
            for j in range(3):
                for h in range(H):
                    ci_box[0] = ci
                    combo_kw = {}
                    if ci == 0:
                        combo_kw = dict(pre_sc=c0_pre_sc, pre_av=c0_pre_av)
                    elif ci == 1:
                        combo_kw = dict(pre_tail=c1_pre_tail)
                    tail = combo(ci, i, j, h, tail, drip, **combo_kw)
                    ci += 1
        for t in tail:
            t()
        while dripq:
            dripq.pop(0)[0]()
        for u in ph3_units(2, dts=(1,)):
            u()
    nc.compile()
    return nc


def kernel(x_btc, x_eth, x_sol, Wq, bq, Wk, bk, Wv, bv, Wo, bo):
    import ml_dtypes
    from concourse.bass_utils import run_bass_kernel_spmd

    if "nc" not in _CACHE:
        _CACHE["nc"] = _build()
    nc = _CACHE["nc"]

    bff = ml_dtypes.bfloat16
    xs = [np.asarray(t, dtype=np.float32) for t in (x_btc, x_eth, x_sol)]
    # fold v-bias and o-bias: out = attn @ Wo + (sum_j bv_j) @ Wo + bo
    bo2 = (np.asarray(bo, np.float64)
           + np.asarray(bv, np.float64).sum(0) @ np.asarray(Wo, np.float64))
    # weight pack [p, (a, ty, kt, f)]: wpack[p, a,ty,kt,f] = W_ty[a, kt*128+p, f]
    wqkv = np.stack([np.asarray(W, np.float32) for W in (Wq, Wk, Wv)], axis=1)
    wpack = np.ascontiguousarray(
        wqkv.reshape(3, 3, 2, 128, D).transpose(3, 0, 1, 2, 4).reshape(128, 3 * 3 * 2 * D)
    ).astype(bff)
    wo_p = np.ascontiguousarray(
        np.asarray(Wo, np.float32).reshape(2, 128, D).transpose(1, 0, 2).reshape(128, 2 * D)
    ).astype(bff)
    bq_p = np.ascontiguousarray(
        np.asarray(bq, np.float32).reshape(3, 2, 128).transpose(2, 0, 1).reshape(128, 6)
    )
    common = {
        "wpack": wpack,
        "wo": wo_p,
        "bqp": bq_p,
        "bo2": bo2.astype(np.float32).astype(bff),
    }
    in_maps = []
    for c in range(N_CORES):
        b, sq = c // 4, c % 4
        # Roll the sequence so this core's query quarter sits at rows [0:512)
        # (the kernel always projects q from rows 0:512).  k/v see the rolled
        # full sequence, which is fine: softmax+sum over the key axis is
        # permutation-invariant.
        xq = np.stack(
            [np.roll(xs[i][b], -sq * SQ, axis=0) for i in range(3)]
        ).astype(bff)
        in_maps.append({"x": np.ascontiguousarray(xq), **common})
    import os
    res = run_bass_kernel_spmd(
        nc, in_maps, core_ids=list(range(N_CORES)),
        trace=bool(os.environ.get("BASS_TRACE")),
    )
    _CACHE["last_res"] = res

    outs = [np.empty((B, S, D), np.float32) for _ in range(3)]
    for c in range(N_CORES):
        b, sq = c // 4, c % 4
        o = res.results[c]["out"]
        for i in range(3):
            outs[i][b, sq * SQ : (sq + 1) * SQ] = o[i]
    return tuple(outs)


if __name__ == "__main__":
    import reference

    inp = reference.setup_inputs()
    inp = {k: np.asarray(v) for k, v in inp.items()}
    got = kernel(**inp)
    exp = reference.reference(**inp)
    for i in range(3):
        g, e = np.asarray(got[i]), np.asarray(exp[i])
        err = np.abs(g - e).max() / np.abs(e).max()
        print(f"out[{i}] rel err {err:.3e}")

